# revision 12
# baseline (speedup 1.0000x reference)
"""Trainium2 Bass kernel for nn_Decoder_52845277610164.

LSTM decoder, B=256 x T=200 recurrence. Pure data parallel over 8 cores
(32 batch rows per core). Everything on-device runs in a transposed layout:
states are [hidden-on-partitions, batch-on-free], so the per-step pointwise
LSTM cell uses full 128-lane DVE/ACT ops and the matmuls keep weights
stationary (bf16, fast-weight-load) with the small batch as the moving
operand.

Device-side reformulation (validated in numpy against the reference):
  - sigmoid-only nonlinearity: tanh(x) = 2*sigmoid(2x) - 1 is folded into the
    weights (gg-gate rows and Wp2 are pre-scaled by 2), and the kernel tracks
    h/2 and c/2 so the 2s-1 corrections collapse into fused
    scalar_tensor_tensor ops.
  - TRACK_ALPHA == 1 and the harness joint limits make the clip provably
    inactive (asserted at build time), so the action feedback becomes
    s = sigmoid(2*a2) and prev_norm = alpha*s + beta folds into the Wih
    prev-columns host-side.
  - per-step constants (z contribution + time features + biases) enter PSUM
    via identity matmuls (engine writes to PSUM would be clobbered by
    matmul accumulation on TRN2).
Outputs are produced on-device in transposed [feature, time*batch] layout and
reordered to [B, T, D] on the host (pure numpy reshape/transpose).
"""

import math
import numpy as np
import ml_dtypes
from contextlib import ExitStack

import concourse.bass as bass
import concourse.tile as tile
from concourse import bacc
from concourse import mybir
from concourse import bass_utils

FP32 = mybir.dt.float32
F32R = mybir.dt.float32r
BF16 = mybir.dt.float16  # 16-bit compute dtype: fp16 (same PE speed as bf16, 4 more mantissa bits)
AF = mybir.ActivationFunctionType
ALU = mybir.AluOpType
BFnp = np.float16

B, T, Z, H, DOF = 256, 200, 64, 256, 12
NL = 13
PD = (NL + 1) * 3          # 42
TBANDS = 6
TD = 2 * TBANDS + 1        # 13
ASC = 0.25
NCORES = 8
BL = B // NCORES           # 32
NM = 8                     # gate row-tiles (4H/128)
PG = 8                     # post-phase timesteps per group
ASSUME_NO_CLIP_DEFAULT = True

LAST_RESULTS = None        # test harness reads exec_time_ns from here


def _time_feats(t_steps):
    t = np.linspace(0.0, 1.0, t_steps, dtype=np.float32)
    feats = [t]
    for k in range(TBANDS):
        f = 2.0 ** k
        feats.append(np.sin(2 * math.pi * f * t))
        feats.append(np.cos(2 * math.pi * f * t))
    return np.stack(feats, axis=-1).astype(np.float32)   # [T, 13]


def _bf(x):
    return np.ascontiguousarray(np.asarray(x, dtype=np.float32).astype(BFnp))


def _f32(x):
    return np.ascontiguousarray(np.asarray(x, dtype=np.float32))


def _host_prep(inp, t_steps):
    """All the weight folds. Returns dict name -> np array (shared across
    cores) plus metadata."""
    f = {k: np.asarray(v, np.float32) for k, v in inp.items()}
    jl, ju, d = f['joint_lower'], f['joint_upper'], f['default_dof_pos']
    jr = (ju - jl) / 2.0
    jm = (ju + jl) / 2.0

    no_clip = bool(np.all(d - ASC >= jl) and np.all(d + ASC <= ju))
    assert no_clip, (
        "joint clip would be active for these inputs; this kernel assumes "
        "TRACK_ALPHA=1 with inactive clip (holds for the harness inputs)")

    Wih, Whh = f['Wih'], f['Whh']
    Wihp = Wih[:, :DOF]
    Wihz = Wih[:, DOF:DOF + Z]
    Wihtf = Wih[:, DOF + Z:]

    alpha = 0.5 / jr
    beta = (d - ASC - jm) / jr
    Wihs = Wihp * alpha[None, :]
    bconst = f['bih'] + f['bhh'] + Wihp @ beta
    tfc = _time_feats(t_steps) @ Wihtf.T                 # [T,1024]

    rs = np.ones(4 * H, np.float32)
    rs[2 * H:3 * H] = 2.0
    perm = np.concatenate([
        np.arange(0, 2 * H),          # i -> m0,m1 ; f -> m2,m3
        np.arange(3 * H, 4 * H),      # o -> m4,m5
        np.arange(2 * H, 3 * H),      # gg -> m6,m7 (rows x2)
    ])
    Whh2p = (2.0 * Whh * rs[:, None])[perm]              # [1024,256]
    Wihsp = (Wihs * rs[:, None])[perm]                   # [1024,12]
    Wihzp = (Wihz * rs[:, None])[perm]                   # [1024,64]
    bconstp = (bconst * rs)[perm]                        # [1024]
    tfcp = (tfc * rs[None, :])[:, perm]                  # [T,1024]

    dat = {}
    # gate weights: WhhT [128, 16*128] bf16, tile (m,k) at col (m*2+k)*128
    WhhT = np.zeros((128, NM * 2 * 128), np.float32)
    for m in range(NM):
        for k in range(2):
            blk = Whh2p[m * 128:(m + 1) * 128, k * 128:(k + 1) * 128]
            WhhT[:, (m * 2 + k) * 128:(m * 2 + k + 1) * 128] = blk.T
    dat['WhhT'] = _bf(WhhT)

    WihsT = np.zeros((DOF, NM * 128), np.float32)
    for m in range(NM):
        WihsT[:, m * 128:(m + 1) * 128] = Wihsp[m * 128:(m + 1) * 128, :].T
    dat['WihsT'] = _bf(WihsT)

    WihzT = np.zeros((Z, NM * 128), np.float32)
    for m in range(NM):
        WihzT[:, m * 128:(m + 1) * 128] = Wihzp[m * 128:(m + 1) * 128, :].T
    dat['WihzT'] = _f32(WihzT)

    bconst_pre = np.zeros((128, NM * BL), np.float32)
    for m in range(NM):
        bconst_pre[:, m * BL:(m + 1) * BL] = bconstp[m * 128:(m + 1) * 128, None]
    dat['bconst_pre'] = bconst_pre

    tfcT = np.zeros((128, NM * t_steps), np.float32)
    for m in range(NM):
        tfcT[:, m * t_steps:(m + 1) * t_steps] = tfcp[:, m * 128:(m + 1) * 128].T
    dat['tfcT'] = _bf(tfcT)

    # policy MLP
    Wp12 = 2.0 * f['Wp1']                                # h2 -> x2
    Wp1T = np.zeros((128, 4 * 128), np.float32)
    for n in range(2):
        for k in range(2):
            blk = Wp12[n * 128:(n + 1) * 128, k * 128:(k + 1) * 128]
            Wp1T[:, (n * 2 + k) * 128:(n * 2 + k + 1) * 128] = blk.T
    dat['Wp1T'] = _bf(Wp1T)
    bp1b = np.zeros((128, 2 * BL), np.float32)
    for n in range(2):
        bp1b[:, n * BL:(n + 1) * BL] = f['bp1'][n * 128:(n + 1) * 128, None]
    dat['bp1b'] = _bf(bp1b)

    Wp2s = 2.0 * f['Wp2']                                # sigmoid trick
    Wp2T = np.zeros((128, 2 * DOF), np.float32)
    for k in range(2):
        Wp2T[:, k * DOF:(k + 1) * DOF] = Wp2s[:, k * 128:(k + 1) * 128].T
    dat['Wp2T'] = _bf(Wp2T)
    dat['bp2sT'] = _f32(2.0 * f['bp2'])[:, None]         # [12,1]

    # init projections (x0.5 for h/2, c/2 convention)
    WhzT = np.zeros((Z, 2 * 128), np.float32)
    WczT = np.zeros((Z, 2 * 128), np.float32)
    for k in range(2):
        WhzT[:, k * 128:(k + 1) * 128] = (0.5 * f['W_hz'][k * 128:(k + 1) * 128]).T
        WczT[:, k * 128:(k + 1) * 128] = (0.5 * f['W_cz'][k * 128:(k + 1) * 128]).T
    dat['WhzT'] = _f32(WhzT)
    dat['WczT'] = _f32(WczT)
    bh0b = np.zeros((128, 2 * BL), np.float32)
    bc0b = np.zeros((128, 2 * BL), np.float32)
    for k in range(2):
        bh0b[:, k * BL:(k + 1) * BL] = (0.5 * f['b_hz'][k * 128:(k + 1) * 128, None])
        bc0b[:, k * BL:(k + 1) * BL] = (0.5 * f['b_cz'][k * 128:(k + 1) * 128, None])
    dat['bh0b'] = bh0b
    dat['bc0b'] = bc0b

    # post phase
    Wo12 = 2.0 * f['Wo1']
    Wo1T = np.zeros((128, 4 * 128), np.float32)
    for n in range(2):
        for k in range(2):
            blk = Wo12[n * 128:(n + 1) * 128, k * 128:(k + 1) * 128]
            Wo1T[:, (n * 2 + k) * 128:(n * 2 + k + 1) * 128] = blk.T
    dat['Wo1T'] = _bf(Wo1T)
    dat['bo1T'] = _f32(f['bo1'].reshape(2, 128).T)       # [128, 2]
    dat['Wo2T'] = _bf(f['Wo2'].T.reshape(2, 128, 3).transpose(1, 0, 2)
                      .reshape(128, 2 * 3))              # tile k at col k*3
    dat['WvT'] = _bf((2.0 * f['Wv']).T.reshape(2, 128, PD)
                     .transpose(1, 0, 2).reshape(128, 2 * PD))
    dat['bvT'] = _f32(f['bv'])[:, None]                  # [42,1]

    dat['WfkT'] = _bf(f['Wfk'].T)                       # [12, 39]
    pm = f['pos_mean'].reshape(-1)
    ps = f['pos_std'].reshape(-1)
    ipsA = 1.0 / ps[:39]
    ipsO = 1.0 / ps[39:]
    dat['gxA_scale'] = _f32(ipsA)[:, None]
    dat['gxA_bias'] = _f32((f['bfk'] - pm[:39]) * ipsA)[:, None]
    dat['gxO_add'] = _f32(f['bo2'] - pm[39:])[:, None]   # DVE: (x + add) * mult
    dat['gxO_mult'] = _f32(ipsO)[:, None]
    dat['jnt_bias'] = _f32(d - ASC)[:, None]             # [12,1]

    dat['id128'] = _bf(np.eye(128, dtype=np.float32))
    return dat


def _build_program(t_steps):
    """Builds the SPMD bass program. Returns (nc, input name->?(shape,dtype),
    output names)."""
    nc = bacc.Bacc("TRN2", target_bir_lowering=False, debug=False,
                   num_devices=NCORES)
    NPG = t_steps // PG
    assert t_steps % PG == 0

    din = {}

    def dram_in(name, shape, dt):
        h = nc.dram_tensor(name, list(shape), dt, kind="ExternalInput")
        din[name] = h.ap()
        return din[name]

    def dram_out(name, shape, dt):
        h = nc.dram_tensor(name, list(shape), dt, kind="ExternalOutput")
        return h.ap()

    zT_d = dram_in('zT', (Z, BL), FP32)
    WhhT_d = dram_in('WhhT', (128, NM * 2 * 128), BF16)
    WihsT_d = dram_in('WihsT', (DOF, NM * 128), BF16)
    WihzT_d = dram_in('WihzT', (Z, NM * 128), FP32)
    bconst_d = dram_in('bconst_pre', (128, NM * BL), FP32)
    tfcT_d = dram_in('tfcT', (128, NM * t_steps), BF16)
    Wp1T_d = dram_in('Wp1T', (128, 4 * 128), BF16)
    bp1b_d = dram_in('bp1b', (128, 2 * BL), BF16)
    Wp2T_d = dram_in('Wp2T', (128, 2 * DOF), BF16)
    bp2sT_d = dram_in('bp2sT', (DOF, 1), FP32)
    WhzT_d = dram_in('WhzT', (Z, 2 * 128), FP32)
    WczT_d = dram_in('WczT', (Z, 2 * 128), FP32)
    bh0b_d = dram_in('bh0b', (128, 2 * BL), FP32)
    bc0b_d = dram_in('bc0b', (128, 2 * BL), FP32)
    Wo1T_d = dram_in('Wo1T', (128, 4 * 128), BF16)
    bo1T_d = dram_in('bo1T', (128, 2), FP32)
    Wo2T_d = dram_in('Wo2T', (128, 2 * 3), BF16)
    WvT_d = dram_in('WvT', (128, 2 * PD), BF16)
    bvT_d = dram_in('bvT', (PD, 1), FP32)
    WfkT_d = dram_in('WfkT', (DOF, 39), BF16)
    gxA_scale_d = dram_in('gxA_scale', (39, 1), FP32)
    gxA_bias_d = dram_in('gxA_bias', (39, 1), FP32)
    gxO_add_d = dram_in('gxO_add', (3, 1), FP32)
    gxO_mult_d = dram_in('gxO_mult', (3, 1), FP32)
    jnt_bias_d = dram_in('jnt_bias', (DOF, 1), FP32)
    id128_d = dram_in('id128', (128, 128), BF16)

    TB32 = t_steps * BL
    gx_d = dram_out('gx', (PD, TB32), FP32)
    jnt_d = dram_out('jnt', (DOF, TB32), BF16)
    act_d = dram_out('act', (DOF, TB32), FP32)
    ls_d = dram_out('ls', (PD, TB32), FP32)

    with tile.TileContext(nc) as tc:
        with ExitStack() as ctx:
            const = ctx.enter_context(tc.tile_pool(name="const", bufs=1))

            def load(dram_ap, shape, dt, _tag=[0]):
                _tag[0] += 1
                t = const.tile(list(shape), dt, tag=f"cin{_tag[0]}")
                nc.sync.dma_start(t[:], dram_ap)
                return t

            zT = load(zT_d, (Z, BL), FP32)
            WhhT = load(WhhT_d, (128, NM * 2 * 128), BF16)
            WihsT = load(WihsT_d, (DOF, NM * 128), BF16)
            WihzT = load(WihzT_d, (Z, NM * 128), FP32)
            bconst_pre = load(bconst_d, (128, NM * BL), FP32)
            tfcT = load(tfcT_d, (128, NM * t_steps), BF16)
            Wp1T = load(Wp1T_d, (128, 4 * 128), BF16)
            bp1b = load(bp1b_d, (128, 2 * BL), BF16)
            Wp2T = load(Wp2T_d, (128, 2 * DOF), BF16)
            bp2sT = load(bp2sT_d, (DOF, 1), FP32)
            WhzT = load(WhzT_d, (Z, 2 * 128), FP32)
            WczT = load(WczT_d, (Z, 2 * 128), FP32)
            bh0b = load(bh0b_d, (128, 2 * BL), FP32)
            bc0b = load(bc0b_d, (128, 2 * BL), FP32)
            Wo1T = load(Wo1T_d, (128, 4 * 128), BF16)
            bo1T = load(bo1T_d, (128, 2), FP32)
            Wo2T = load(Wo2T_d, (128, 2 * 3), BF16)
            WvT = load(WvT_d, (128, 2 * PD), BF16)
            bvT = load(bvT_d, (PD, 1), FP32)
            WfkT = load(WfkT_d, (DOF, 39), BF16)
            gxA_scale = load(gxA_scale_d, (39, 1), FP32)
            gxA_bias = load(gxA_bias_d, (39, 1), FP32)
            gxO_add = load(gxO_add_d, (3, 1), FP32)
            gxO_mult = load(gxO_mult_d, (3, 1), FP32)
            jnt_bias = load(jnt_bias_d, (DOF, 1), FP32)
            id128 = load(id128_d, (128, 128), BF16)

            # persistent state buffers
            h2ring = const.tile([128, (t_steps + 1) * 2 * BL], BF16, tag='h2ring')
            sring = const.tile([DOF, t_steps * BL], BF16, tag='sring')
            zc_sb = const.tile([128, NM * BL], BF16, tag='zc_sb')
            sgbuf = const.tile([PD, t_steps * BL], FP32, tag='sgbuf')
            jntbuf = const.tile([DOF, t_steps * BL], BF16, tag='jntbuf')

            C_init = const.tile([128, 2 * BL], BF16, tag='C_init')

            # ---------------- pre phase ----------------
            with tc.tile_pool(name="prepsum", bufs=1,
                              space=bass.MemorySpace.PSUM) as prepsum:
                zc_ps = prepsum.tile([128, NM * BL], FP32)
                for m in range(NM):
                    nc.tensor.matmul(
                        zc_ps[:, m * BL:(m + 1) * BL],
                        WihzT[:, m * 128:(m + 1) * 128],
                        zT[:],
                        start=True, stop=True)
                nc.vector.tensor_add(zc_sb[:], zc_ps[:], bconst_pre[:])

                h0_ps = prepsum.tile([128, 2 * BL], FP32)
                for k in range(2):
                    nc.tensor.matmul(
                        h0_ps[:, k * BL:(k + 1) * BL],
                        WhzT[:, k * 128:(k + 1) * 128],
                        zT[:], start=True, stop=True)
                nc.vector.tensor_add(h2ring[:, 0:2 * BL], h0_ps[:], bh0b[:])

                c0_ps = prepsum.tile([128, 2 * BL], FP32)
                for k in range(2):
                    nc.tensor.matmul(
                        c0_ps[:, k * BL:(k + 1) * BL],
                        WczT[:, k * 128:(k + 1) * 128],
                        zT[:], start=True, stop=True)
                nc.vector.tensor_add(C_init[:], c0_ps[:], bc0b[:])

            s_init = const.tile([DOF, BL], BF16, tag='s_init')
            nc.vector.memset(s_init[:], 0.5)

            # tfcT viewed [128, NM, t] for per-step broadcast
            tfc_v = tfcT[:].rearrange("p (m t) -> p m t", m=NM)

            # ---------------- recurrence ----------------
            with tc.tile_pool(name="gpsum", bufs=2,
                              space=bass.MemorySpace.PSUM) as gpsum, \
                 tc.tile_pool(name="a1psum", bufs=2,
                              space=bass.MemorySpace.PSUM) as a1psum, \
                 tc.tile_pool(name="a2psum", bufs=2,
                              space=bass.MemorySpace.PSUM) as a2psum, \
                 tc.tile_pool(name="cell", bufs=3) as cell:

                C_prev = C_init
                s_prev = s_init
                for n in range(t_steps):
                    hslice = h2ring[:, n * 64:(n + 1) * 64]

                    # constants into PSUM via identity matmuls
                    tfb = cell.tile([128, NM * BL], BF16, tag="tfb")
                    nc.vector.scalar_tensor_tensor(
                        tfb[:].rearrange("p (m b) -> p m b", m=NM),
                        tfc_v[:, :, n:n + 1].broadcast_to([128, NM, BL]),
                        1.0,
                        zc_sb[:].rearrange("p (m b) -> p m b", m=NM),
                        op0=ALU.mult, op1=ALU.add)

                    gp = gpsum.tile([128, NM * BL], FP32, tag="gp")
                    for m in range(NM):
                        reg = gp[:, m * BL:(m + 1) * BL]
                        nc.tensor.matmul(reg, id128[:],
                                         tfb[:, m * BL:(m + 1) * BL],
                                         start=True, stop=False)
                        for k in range(2):
                            nc.tensor.matmul(
                                reg,
                                WhhT[:, (m * 2 + k) * 128:(m * 2 + k + 1) * 128],
                                hslice[:, k * BL:(k + 1) * BL],
                                start=False, stop=False)
                        nc.tensor.matmul(
                            reg, WihsT[:, m * 128:(m + 1) * 128], s_prev[:],
                            start=False, stop=True)

                    sig4 = cell.tile([128, NM * BL], BF16, tag="sig4")
                    nc.scalar.activation(sig4[:], gp[:], AF.Sigmoid)
                    # layout: [0:64]=si, [64:128]=sf, [128:192]=so, [192:256]=s2g
                    m1 = cell.tile([128, 2 * BL], BF16, tag="m1")
                    nc.vector.tensor_mul(m1[:], sig4[:, 64:128], C_prev[:])
                    m2 = cell.tile([128, 2 * BL], BF16, tag="m2")
                    nc.vector.scalar_tensor_tensor(
                        m2[:], sig4[:, 192:256], 0.5, sig4[:, 0:64],
                        op0=ALU.subtract, op1=ALU.mult)
                    C_new = cell.tile([128, 2 * BL], BF16, tag="C")
                    nc.vector.tensor_add(C_new[:], m1[:], m2[:])
                    s4c = cell.tile([128, 2 * BL], BF16, tag="s4c")
                    nc.scalar.activation(s4c[:], C_new[:], AF.Sigmoid, scale=4.0)
                    out_h = h2ring[:, (n + 1) * 64:(n + 2) * 64]
                    nc.vector.scalar_tensor_tensor(
                        out_h, s4c[:], 0.5, sig4[:, 128:192],
                        op0=ALU.subtract, op1=ALU.mult)

                    # tail: a1 = relu(Wp1*h2 + bp1); s = sigmoid(Wp2s*a1 + bp2s)
                    a1p = a1psum.tile([128, 2 * BL], FP32, tag="a1p")
                    for r in range(2):
                        reg = a1p[:, r * BL:(r + 1) * BL]
                        nc.tensor.matmul(reg, id128[:],
                                         bp1b[:, r * BL:(r + 1) * BL],
                                         start=True, stop=False)
                        for k in range(2):
                            nc.tensor.matmul(
                                reg,
                                Wp1T[:, (r * 2 + k) * 128:(r * 2 + k + 1) * 128],
                                out_h[:, k * BL:(k + 1) * BL],
                                start=False, stop=(k == 1))
                    a1r = cell.tile([128, 2 * BL], BF16, tag="a1r")
                    nc.vector.tensor_scalar(a1r[:], a1p[:], 0.0, None,
                                            op0=ALU.max)
                    a2p = a2psum.tile([DOF, BL], FP32, tag="a2p")
                    for k in range(2):
                        nc.tensor.matmul(
                            a2p[:], Wp2T[:, k * DOF:(k + 1) * DOF],
                            a1r[:, k * BL:(k + 1) * BL],
                            start=(k == 0), stop=(k == 1))
                    s_out = sring[:, n * BL:(n + 1) * BL]
                    nc.scalar.activation(s_out, a2p[:], AF.Sigmoid,
                                         bias=bp2sT[:])
                    s_prev = s_out
                    C_prev = C_new

            # ---------------- post phase ----------------
            h2v = h2ring[:].rearrange("p (t k b) -> p t k b", k=2, b=BL)

            # joints (fp32, also FK input) in 4 chunks
            CH = t_steps * BL // 4
            for c in range(4):
                sl = slice(c * CH, (c + 1) * CH)
                nc.vector.tensor_scalar(
                    jntbuf[:, sl], sring[:, sl], 0.5, jnt_bias[:],
                    op0=ALU.mult, op1=ALU.add)

            with tc.tile_pool(name="postpool", bufs=3) as pp, \
                 tc.tile_pool(name="o1ps", bufs=2,
                              space=bass.MemorySpace.PSUM) as o1ps, \
                 tc.tile_pool(name="o2ps", bufs=2,
                              space=bass.MemorySpace.PSUM) as o2ps, \
                 tc.tile_pool(name="vps", bufs=2,
                              space=bass.MemorySpace.PSUM) as vps:

                for c in range(4):
                    sl = slice(c * CH, (c + 1) * CH)
                    at = pp.tile([DOF, CH], FP32, tag="act")
                    nc.vector.tensor_scalar(at[:], sring[:, sl], 2.0, -1.0,
                                            op0=ALU.mult, op1=ALU.add)
                    nc.sync.dma_start(act_d[:, sl], at[:])
                    nc.sync.dma_start(jnt_d[:, sl], jntbuf[:, sl])

                NG = PG * BL     # 256
                for g in range(t_steps // PG):
                    t0 = g * PG
                    # rhs views: [128, PG, BL] for each k
                    rhs = [h2v[:, t0 + 1:t0 + 1 + PG, k, :] for k in range(2)]
                    o1p = o1ps.tile([128, 2 * NG], FP32, tag="o1p")
                    for r in range(2):
                        for k in range(2):
                            nc.tensor.matmul(
                                o1p[:, r * NG:(r + 1) * NG],
                                Wo1T[:, (r * 2 + k) * 128:(r * 2 + k + 1) * 128],
                                rhs[k], start=(k == 0), stop=(k == 1))
                    o1r = pp.tile([128, 2 * NG], BF16, tag="o1r")
                    # split relu across DVE / ACT
                    nc.vector.tensor_scalar(
                        o1r[:, 0:NG], o1p[:, 0:NG], bo1T[:, 0:1], 0.0,
                        op0=ALU.add, op1=ALU.max)
                    nc.scalar.activation(
                        o1r[:, NG:2 * NG], o1p[:, NG:2 * NG], AF.Relu,
                        bias=bo1T[:, 1:2])
                    o2p = o2ps.tile([3, NG], FP32, tag="o2p")
                    for k in range(2):
                        nc.tensor.matmul(
                            o2p[:], Wo2T[:, k * 3:(k + 1) * 3],
                            o1r[:, k * NG:(k + 1) * NG],
                            start=(k == 0), stop=(k == 1))
                    gxo = pp.tile([3, NG], FP32, tag="gxo")
                    nc.vector.tensor_scalar(gxo[:], o2p[:], gxO_add[:],
                                            gxO_mult[:], op0=ALU.add,
                                            op1=ALU.mult)
                    nc.sync.dma_start(gx_d[39:42, t0 * BL:(t0 + PG) * BL],
                                      gxo[:])

                    vp = vps.tile([PD, NG], FP32, tag="vp")
                    for k in range(2):
                        nc.tensor.matmul(
                            vp[:], WvT[:, k * PD:(k + 1) * PD],
                            rhs[k], start=(k == 0), stop=(k == 1))
                    nc.scalar.activation(sgbuf[:, t0 * BL:(t0 + PG) * BL],
                                         vp[:], AF.Sigmoid, bias=bvT[:])

            with tc.tile_pool(name="fkpool", bufs=3) as fp, \
                 tc.tile_pool(name="fkps", bufs=2,
                              space=bass.MemorySpace.PSUM) as fkps:
                NG = PG * BL
                for g in range(t_steps // PG):
                    t0 = g * PG
                    fk = fkps.tile([39, NG], FP32, tag="fk")
                    nc.tensor.matmul(fk[:], WfkT[:],
                                     jntbuf[:, t0 * BL:(t0 + PG) * BL],
                                     start=True, stop=True)
                    gxa = fp.tile([39, NG], FP32, tag="gxa")
                    nc.scalar.activation(gxa[:], fk[:], AF.Identity,
                                         bias=gxA_bias[:], scale=gxA_scale[:])
                    nc.sync.dma_start(gx_d[0:39, t0 * BL:(t0 + PG) * BL],
                                      gxa[:])

                # log pass (single ACT table switch)
                ls_bias = fp.tile([PD, 1], FP32, tag="lsb")
                nc.vector.memset(ls_bias[:], 0.05)
                LCH = t_steps * BL // 4
                for c in range(4):
                    sl = slice(c * LCH, (c + 1) * LCH)
                    lt = fp.tile([PD, LCH], FP32, tag="ls")
                    nc.scalar.activation(lt[:], sgbuf[:, sl], AF.Ln,
                                         bias=ls_bias[:], scale=0.45)
                    nc.sync.dma_start(ls_d[:, sl], lt[:])

    nc.compile()
    return nc


_CACHE = {}


def _get_program(t_steps):
    if t_steps not in _CACHE:
        _CACHE[t_steps] = _build_program(t_steps)
    return _CACHE[t_steps]


def make_in_maps(inputs, t_steps):
    dat = _host_prep(inputs, t_steps)
    z = np.asarray(inputs['z'], np.float32)
    in_maps = []
    for c in range(NCORES):
        m = {k: v for k, v in dat.items()}
        m['zT'] = _f32(z[c * BL:(c + 1) * BL, :].T)
        in_maps.append(m)
    return in_maps


def assemble_outputs(per_core, t_steps):
    """per_core: list of dicts name->np array in device layout."""
    gx = np.zeros((B, t_steps, PD), np.float32)
    jnt = np.zeros((B, t_steps, DOF), np.float32)
    act = np.zeros((B, t_steps, DOF), np.float32)
    ls = np.zeros((B, t_steps, PD), np.float32)
    for c in range(NCORES):
        r = per_core[c]
        bsl = slice(c * BL, (c + 1) * BL)
        # device layout [D, t*BL] -> [BL, t, D]
        gx[bsl] = np.asarray(r['gx'], np.float32).reshape(
            PD, t_steps, BL).transpose(2, 1, 0)
        jnt[bsl] = np.asarray(r['jnt'], np.float32).reshape(
            DOF, t_steps, BL).transpose(2, 1, 0)
        act[bsl] = np.asarray(r['act'], np.float32).reshape(
            DOF, t_steps, BL).transpose(2, 1, 0)
        ls[bsl] = np.asarray(r['ls'], np.float32).reshape(
            PD, t_steps, BL).transpose(2, 1, 0)
    return gx, jnt, act, ls


def run_on_hw(inputs, t_steps=T):
    global LAST_RESULTS
    import os
    in_maps = make_in_maps(inputs, t_steps)
    nc = _get_program(t_steps)
    trace = bool(os.environ.get('BASS_TRACE'))
    res = bass_utils.run_bass_kernel_spmd(
        nc, in_maps, core_ids=list(range(NCORES)), trace=trace)
    LAST_RESULTS = res
    return assemble_outputs(res.results, t_steps)


def kernel(**inputs):
    return run_on_hw(inputs, T)
